# revision 16
# baseline (speedup 1.0000x reference)
"""Trainium2 Bass kernel for per-clique cosine-similarity attention over params.

Computation (per clique c of 64): w = softmax(cos_sim(x_c)), out_c = w @ params_c
with x_c [16, 256], params_c [16, 65536].

Strategy: shard the clique axis across 8 cores (8 cliques/core). Per core the
8 cliques * 16 members = exactly 128 SBUF partitions. The attention front-end
runs once per core on a [128, 256] tile:
  - normalize rows (x / |x|), transpose via PE, gram matrix G = Xh^T Xh [128,128]
  - A = exp(G) on the 8 diagonal 16x16 blocks, zero elsewhere (block-diag,
    symmetric) -> A is directly usable as matmul lhsT for ALL cliques at once
  - softmax row-normalization folds into the PSUM->SBUF copy as a per-partition
    scale 1/rowsum(A)
Then stream params [128, 65536] through SBUF in chunks: matmul (N=512 slices)
against stationary A, scaled-copy to SBUF, DMA out. Memory-bound: ~64 MiB of
HBM traffic per core.
"""

import os
import sys
from contextlib import ExitStack

import numpy as np

sys.path.insert(0, "/opt/trn_rl_repo")

import concourse.bacc as bacc
import concourse.bass as bass
import concourse.mybir as mybir
import concourse.tile as tile
from concourse.bass_utils import run_bass_kernel_spmd
from concourse.masks import make_identity

C, S, D, P = 64, 16, 256, 65536
NCORES = 8
CPM = C // NCORES          # cliques per core
ROWS = CPM * S             # 128 partitions
CHUNK = 8192               # params free-dim elements per DMA chunk
NSUB = CHUNK // 512        # matmuls per chunk (N=512 = one PSUM bank fp32)

FP32 = mybir.dt.float32
AF = mybir.ActivationFunctionType


def _kernel_body(ctx, tc, reps, prm, mask, out, repeat=1, chunk=CHUNK,
                 in_bufs=2, out_bufs=2, ps_bufs=4, out_engine="sync",
                 dma_split=1):
    nc = tc.nc

    consts = ctx.enter_context(tc.tile_pool(name="consts", bufs=1))
    fe = ctx.enter_context(tc.tile_pool(name="fe", bufs=1))

    ident = consts.tile([128, 128], FP32)
    make_identity(nc, ident[:])

    # ---- front-end: build block-diagonal A = exp(gram) and row scales ----
    x = fe.tile([128, D], FP32)
    nc.sync.dma_start(out=x[:], in_=reps[:])

    xsq = fe.tile([128, D], FP32)
    ss = fe.tile([128, 1], FP32)
    nc.scalar.activation(xsq[:], x[:], AF.Square, accum_out=ss[:])
    norm = fe.tile([128, 1], FP32)
    nc.scalar.sqrt(norm[:], ss[:])
    rn = fe.tile([128, 1], FP32)
    nc.vector.reciprocal(rn[:], norm[:])
    xh = fe.tile([128, D], FP32)
    nc.scalar.mul(xh[:], x[:], rn[:])

    msk = fe.tile([128, 128], FP32)
    nc.sync.dma_start(out=msk[:], in_=mask[:])

    A = fe.tile([128, 128], FP32)

    with tc.tile_pool(name="fe_ps", bufs=2, space="PSUM") as fe_ps:
        tsb = []
        for k in range(2):
            tps = fe_ps.tile([128, 128], FP32, tag="tp")
            nc.tensor.transpose(tps[:], xh[:, 128 * k : 128 * (k + 1)], ident[:])
            t = fe.tile([128, 128], FP32, tag=f"tsb{k}")
            nc.vector.tensor_copy(t[:], tps[:])
            tsb.append(t)

        simps = fe_ps.tile([128, 128], FP32, tag="sim")
        for k in range(2):
            nc.tensor.matmul(
                simps[:], tsb[k][:], tsb[k][:], start=(k == 0), stop=(k == 1)
            )
        # exp of ALL pairwise cosine sims (all in [-1,1], no overflow), then
        # zero the cross-clique blocks -> block-diagonal symmetric A.
        nc.scalar.activation(A[:], simps[:], AF.Exp)
        nc.vector.tensor_mul(A[:], A[:], msk[:])

    r = fe.tile([128, 1], FP32)
    nc.vector.reduce_sum(r[:], A[:], axis=mybir.AxisListType.X)
    rr = fe.tile([128, 1], FP32)
    nc.vector.reciprocal(rr[:], r[:])

    # ---- streaming loop: out = (A @ params) * rr ----
    io = ctx.enter_context(tc.tile_pool(name="io", bufs=2))
    ps = ctx.enter_context(tc.tile_pool(name="mmps", bufs=ps_bufs, space="PSUM"))

    out_eng = {"sync": nc.sync, "scalar": nc.scalar, "gpsimd": nc.gpsimd}[out_engine]
    nsub = chunk // 512
    half = chunk // dma_split
    for _rep in range(repeat):
        for ci in range(P // chunk):
            off = ci * chunk
            pin = io.tile([128, chunk], FP32, tag="pin", bufs=in_bufs)
            for h in range(dma_split):
                nc.sync.dma_start(
                    out=pin[:, h * half : (h + 1) * half],
                    in_=prm[:, off + h * half : off + (h + 1) * half],
                )
            pout = io.tile([128, chunk], FP32, tag="pout", bufs=out_bufs)
            for n in range(nsub):
                mm = ps.tile([128, 512], FP32, tag="mm")
                nc.tensor.matmul(
                    mm[:], A[:], pin[:, 512 * n : 512 * (n + 1)], start=True, stop=True
                )
                nc.vector.tensor_scalar_mul(
                    pout[:, 512 * n : 512 * (n + 1)], mm[:], rr[:]
                )
            for h in range(dma_split):
                out_eng.dma_start(
                    out=out[:, off + h * half : off + (h + 1) * half],
                    in_=pout[:, h * half : (h + 1) * half],
                )


_NC_CACHE = {}


def _build_nc(repeat=1, **cfg):
    key = (repeat, tuple(sorted(cfg.items())))
    if key in _NC_CACHE:
        return _NC_CACHE[key]
    nc = bacc.Bacc(
        "TRN2",
        target_bir_lowering=False,
        debug=False,
        num_devices=NCORES,
    )
    reps = nc.dram_tensor("reps", [ROWS, D], FP32, kind="ExternalInput")
    prm = nc.dram_tensor("prm", [ROWS, P], FP32, kind="ExternalInput")
    mask = nc.dram_tensor("mask", [128, 128], FP32, kind="ExternalInput")
    out = nc.dram_tensor("out", [ROWS, P], FP32, kind="ExternalOutput")
    with tile.TileContext(nc) as tc:
        with ExitStack() as ctx:
            _kernel_body(
                ctx, tc, reps.ap(), prm.ap(), mask.ap(), out.ap(), repeat=repeat,
                **cfg,
            )
    nc.compile()
    _NC_CACHE[key] = nc
    return nc


def run_sharded(dimension_reps, params, trace=False):
    """Run the SPMD kernel; returns (full_output, BassKernelResults)."""
    reps = np.ascontiguousarray(np.asarray(dimension_reps, dtype=np.float32))
    prm = np.ascontiguousarray(np.asarray(params, dtype=np.float32))
    assert reps.shape == (C, S, D) and prm.shape == (C, S, P)

    nc = _build_nc()
    blockmask = np.kron(np.eye(CPM, dtype=np.float32), np.ones((S, S), np.float32))
    in_maps = []
    for m in range(NCORES):
        sl = slice(m * CPM, (m + 1) * CPM)
        in_maps.append(
            {
                "reps": reps[sl].reshape(ROWS, D),
                "prm": prm[sl].reshape(ROWS, P),
                "mask": blockmask,
            }
        )
    res = run_bass_kernel_spmd(nc, in_maps, list(range(NCORES)), trace=trace)
    outs = [res.results[m]["out"].reshape(CPM, S, P) for m in range(NCORES)]
    return np.concatenate(outs, axis=0), res


def kernel(dimension_reps, params):
    full, _ = run_sharded(dimension_reps, params, trace=False)
    return full


# revision 19
# speedup vs baseline: 23.9165x; 23.9165x over previous
"""Trainium2 Bass kernel for per-clique cosine-similarity attention over params.

Computation (per clique c of 64): w = softmax(cos_sim(x_c)), out_c = w @ params_c
with x_c [16, 256], params_c [16, 65536].

Strategy: shard the clique axis across 8 cores (8 cliques/core). Per core the
8 cliques * 16 members = exactly 128 SBUF partitions. The attention front-end
runs once per core on a [128, 256] tile:
  - normalize rows (x / |x|), transpose via PE, gram matrix G = Xh^T Xh [128,128]
  - A = exp(G) on the 8 diagonal 16x16 blocks, zero elsewhere (block-diag,
    symmetric) -> A is directly usable as matmul lhsT for ALL cliques at once
  - softmax row-normalization folds into the PSUM->SBUF copy as a per-partition
    scale 1/rowsum(A)
Then stream params [128, 65536] through SBUF in chunks: matmul (N=512 slices)
against stationary A, scaled-copy to SBUF, DMA out. Memory-bound: ~64 MiB of
HBM traffic per core.
"""

import os
import sys
from contextlib import ExitStack

import numpy as np

sys.path.insert(0, "/opt/trn_rl_repo")

import concourse.bacc as bacc
import concourse.bass as bass
import concourse.mybir as mybir
import concourse.tile as tile
from concourse.bass_utils import run_bass_kernel_spmd
from concourse.masks import make_identity

C, S, D, P = 64, 16, 256, 65536
NCORES = 8
CPM = C // NCORES          # cliques per core
ROWS = CPM * S             # 128 partitions
CHUNK = 8192               # params free-dim elements per DMA chunk
NSUB = CHUNK // 512        # matmuls per chunk (N=512 = one PSUM bank fp32)

FP32 = mybir.dt.float32
AF = mybir.ActivationFunctionType


def _kernel_body(ctx, tc, reps, prm, mask, out, repeat=1, chunk=CHUNK,
                 in_bufs=2, out_bufs=2, ps_bufs=4, out_engine="sync",
                 dma_split=1, hw_loop=0):
    nc = tc.nc

    consts = ctx.enter_context(tc.tile_pool(name="consts", bufs=1))
    fe = ctx.enter_context(tc.tile_pool(name="fe", bufs=1))

    ident = consts.tile([128, 128], FP32)
    make_identity(nc, ident[:])

    # ---- front-end: build block-diagonal A = exp(gram) and row scales ----
    x = fe.tile([128, D], FP32)
    nc.sync.dma_start(out=x[:], in_=reps[:])

    xsq = fe.tile([128, D], FP32)
    ss = fe.tile([128, 1], FP32)
    nc.scalar.activation(xsq[:], x[:], AF.Square, accum_out=ss[:])
    norm = fe.tile([128, 1], FP32)
    nc.scalar.sqrt(norm[:], ss[:])
    rn = fe.tile([128, 1], FP32)
    nc.vector.reciprocal(rn[:], norm[:])
    xh = fe.tile([128, D], FP32)
    nc.scalar.mul(xh[:], x[:], rn[:])

    msk = fe.tile([128, 128], FP32)
    nc.sync.dma_start(out=msk[:], in_=mask[:])

    A = fe.tile([128, 128], FP32)

    with tc.tile_pool(name="fe_ps", bufs=2, space="PSUM") as fe_ps:
        tsb = []
        for k in range(2):
            tps = fe_ps.tile([128, 128], FP32, tag="tp")
            nc.tensor.transpose(tps[:], xh[:, 128 * k : 128 * (k + 1)], ident[:])
            t = fe.tile([128, 128], FP32, tag=f"tsb{k}")
            nc.vector.tensor_copy(t[:], tps[:])
            tsb.append(t)

        simps = fe_ps.tile([128, 128], FP32, tag="sim")
        for k in range(2):
            nc.tensor.matmul(
                simps[:], tsb[k][:], tsb[k][:], start=(k == 0), stop=(k == 1)
            )
        # exp of ALL pairwise cosine sims (all in [-1,1], no overflow), then
        # zero the cross-clique blocks -> block-diagonal symmetric A.
        nc.scalar.activation(A[:], simps[:], AF.Exp)
        nc.vector.tensor_mul(A[:], A[:], msk[:])

    r = fe.tile([128, 1], FP32)
    nc.vector.reduce_sum(r[:], A[:], axis=mybir.AxisListType.X)
    rr = fe.tile([128, 1], FP32)
    nc.vector.reciprocal(rr[:], r[:])

    # ---- streaming loop: out = (A @ params) * rr ----
    io = ctx.enter_context(tc.tile_pool(name="io", bufs=2))
    ps = ctx.enter_context(tc.tile_pool(name="mmps", bufs=ps_bufs, space="PSUM"))

    out_eng = {"sync": nc.sync, "scalar": nc.scalar, "gpsimd": nc.gpsimd}[out_engine]
    nsub = chunk // 512
    half = chunk // dma_split

    def stream_once():
        for ci in range(P // chunk):
            off = ci * chunk
            pin = io.tile([128, chunk], FP32, tag="pin", bufs=in_bufs)
            for h in range(dma_split):
                nc.sync.dma_start(
                    out=pin[:, h * half : (h + 1) * half],
                    in_=prm[:, off + h * half : off + (h + 1) * half],
                )
            pout = io.tile([128, chunk], FP32, tag="pout", bufs=out_bufs)
            for n in range(nsub):
                mm = ps.tile([128, 512], FP32, tag="mm")
                nc.tensor.matmul(
                    mm[:], A[:], pin[:, 512 * n : 512 * (n + 1)], start=True, stop=True
                )
                nc.vector.tensor_scalar_mul(
                    pout[:, 512 * n : 512 * (n + 1)], mm[:], rr[:]
                )
            for h in range(dma_split):
                out_eng.dma_start(
                    out=out[:, off + h * half : off + (h + 1) * half],
                    in_=pout[:, h * half : (h + 1) * half],
                )

    if hw_loop > 1:
        with tc.For_i(0, hw_loop, 1):
            stream_once()
    for _rep in range(repeat):
        stream_once()


_NC_CACHE = {}


def _build_nc(repeat=1, **cfg):
    key = (repeat, tuple(sorted(cfg.items())))
    if key in _NC_CACHE:
        return _NC_CACHE[key]
    nc = bacc.Bacc(
        "TRN2",
        target_bir_lowering=False,
        debug=False,
        num_devices=NCORES,
    )
    reps = nc.dram_tensor("reps", [ROWS, D], FP32, kind="ExternalInput")
    prm = nc.dram_tensor("prm", [ROWS, P], FP32, kind="ExternalInput")
    mask = nc.dram_tensor("mask", [128, 128], FP32, kind="ExternalInput")
    out = nc.dram_tensor("out", [ROWS, P], FP32, kind="ExternalOutput")
    with tile.TileContext(nc) as tc:
        with ExitStack() as ctx:
            _kernel_body(
                ctx, tc, reps.ap(), prm.ap(), mask.ap(), out.ap(), repeat=repeat,
                **cfg,
            )
    nc.compile()
    _NC_CACHE[key] = nc
    return nc


def run_sharded(dimension_reps, params, trace=False):
    """Run the SPMD kernel; returns (full_output, BassKernelResults)."""
    reps = np.ascontiguousarray(np.asarray(dimension_reps, dtype=np.float32))
    prm = np.ascontiguousarray(np.asarray(params, dtype=np.float32))
    assert reps.shape == (C, S, D) and prm.shape == (C, S, P)

    nc = _build_nc()
    blockmask = np.kron(np.eye(CPM, dtype=np.float32), np.ones((S, S), np.float32))
    in_maps = []
    for m in range(NCORES):
        sl = slice(m * CPM, (m + 1) * CPM)
        in_maps.append(
            {
                "reps": reps[sl].reshape(ROWS, D),
                "prm": prm[sl].reshape(ROWS, P),
                "mask": blockmask,
            }
        )
    res = run_bass_kernel_spmd(nc, in_maps, list(range(NCORES)), trace=trace)
    outs = [res.results[m]["out"].reshape(CPM, S, P) for m in range(NCORES)]
    return np.concatenate(outs, axis=0), res


def kernel(dimension_reps, params):
    full, _ = run_sharded(dimension_reps, params, trace=False)
    return full
